# revision 13
# baseline (speedup 1.0000x reference)
"""CBOW model (embedding gather -> mean -> logits -> softmax) on 8 Trainium2
NeuronCores.

Sharding strategy (model/vocab parallel, per the hint):
  - W1 and W2 are both sharded along the vocab axis: core m owns W1 rows
    [m*12500, (m+1)*12500) (fp16, plus an appended zero row) and W2 columns
    [m*12500, (m+1)*12500) (fp16, padded to 12544 with zero columns).
  - Gather: every core looks up ALL 2048x10 indices against its own W1 shard
    with ONE dma_gather per batch tile (transpose mode -> data lands already
    [D, ctx*batch]); out-of-shard indices are remapped (host-side) to the
    zero row, so per-core context sums are partial sums.  A DVE strided
    reduce gives the partial transposed hidden [D, 128] per tile; chunked
    AllReduces(add) give every core the full [D, 2048] hidden.
  - Softmax in a SINGLE matmul/exp pass: for each batch tile, matmul chunks
    produce f32 logits in PSUM; ACT computes exp -> SBUF (bf16, kept
    resident) with fused per-row partial sums (accum_out); DVE/ACT copy the
    f32 logits to f16 and DMA them out.  Per 2-tile chunk the row sums are
    AllReduced(add) across cores; 1/Z comes from DVE reciprocal, and the
    resident exp values are scaled in place (DVE 4x tensor_scalar) and
    DMA'd out as the softmax (bf16) on the ACT HWDGE ring.  No second
    matmul pass and no second exp pass.  Max-subtraction is unnecessary:
    |logit| < 40 always, exp fits bf16 comfortably.
"""

import os

import numpy as np

import concourse.bass as bass
import concourse.mybir as mybir
import concourse.tile as tile
from concourse import bacc
import concourse.bass_utils as bass_utils

# debug knobs (bisection); defaults are the shipping configuration
USE_DMA_GATHER = os.environ.get("K_GATHER", "1") == "1"
SOFT_RING_SCALAR = os.environ.get("K_SOFTRING", "1") == "1"
GATHER_SINGLE_PACKET = os.environ.get("K_SINGLEPKT", "0") == "1"
GATHER_TRANSPOSE = os.environ.get("K_GATHERT", "1") == "1"

# Problem shape (hardcoded; matches the CBOW reference).
V = 100000      # vocab
D = 128         # embed dim
B = 2048        # batch
C = 10          # context positions
M = 8           # cores
S = V // M      # vocab shard per core = 12500
SP = 12544      # shard padded so every matmul chunk is >= 256 wide
P = 128         # partitions
BT = B // P     # batch tiles = 16
MMN = 512       # max moving free dim per matmul (one PSUM bank, f32)
GRP = 2048      # vocab columns per PSUM group (4 banks)

F32 = mybir.dt.float32
F16 = mybir.dt.float16
BF16 = mybir.dt.bfloat16
I16 = mybir.dt.int16
AF = mybir.ActivationFunctionType

# (start, width) vocab-column groups per core; width <= GRP.
GROUPS = [(g0, min(GRP, SP - g0)) for g0 in range(0, SP, GRP)]
# groups whose PSUM->SBUF logits cast runs on the scalar engine instead of
# DVE, to balance the two engines (ACT also does every group's exp).
ACT_COPY_GROUPS = {len(GROUPS) - 1, len(GROUPS) - 2}
# batch tiles per hidden-AllReduce chunk: a 1-tile first chunk minimizes the
# exposed pipeline head; later gathers/AllReduces overlap pass-1 compute.
CHUNKS = [1, 3, 4, 4, 4]
NCH = len(CHUNKS)
CH_START = [sum(CHUNKS[:k]) for k in range(NCH)]
# softmax row sums are AllReduced per SCH batch tiles so the resident exp
# tiles can be scaled + written out while later tiles still compute.
SCH = 2
NSC = BT // SCH


def build_nc(n_cores: int = M):
    nc = bacc.Bacc("TRN2", target_bir_lowering=False, debug=False,
                   num_devices=n_cores)

    w1s = nc.dram_tensor("w1s", [S + 1, D], F16, kind="ExternalInput")
    w2s = nc.dram_tensor("w2s", [P, SP], F16, kind="ExternalInput")
    if USE_DMA_GATHER:
        idx16 = nc.dram_tensor("idx16", [P, BT * C * (P // 16)], I16,
                               kind="ExternalInput")
    else:
        idxs = nc.dram_tensor("idxs", [P, BT * C], mybir.dt.int32,
                              kind="ExternalInput")
    logits_s = nc.dram_tensor("logits_s", [B, S], F16, kind="ExternalOutput")
    soft_s = nc.dram_tensor("soft_s", [B, S], BF16, kind="ExternalOutput")

    rg = [list(range(n_cores))]
    IPT = C * (P // 16)  # idx16 columns per batch tile = 80

    with tile.TileContext(nc) as tc:
        with tc.tile_pool(name="sbuf", bufs=1) as sbuf, \
             tc.tile_pool(name="gathp", bufs=4) as gathp, \
             tc.tile_pool(name="hidp", bufs=2) as hidp, \
             tc.tile_pool(name="stagp", bufs=2) as stagp, \
             tc.tile_pool(name="ebufp", bufs=4) as ebufp, \
             tc.tile_pool(name="psum", bufs=2, space="PSUM") as psum, \
             tc.tile_pool(name="dram", bufs=1, space="DRAM") as dram:
            if USE_DMA_GATHER:
                idx_sb = sbuf.tile([P, BT * IPT], I16)
                nc.sync.dma_start(out=idx_sb[:], in_=idx16[:])
            else:
                idx_sb = sbuf.tile([P, BT * C], mybir.dt.int32)
                nc.sync.dma_start(out=idx_sb[:], in_=idxs[:])
            if not (USE_DMA_GATHER and GATHER_TRANSPOSE):
                from concourse.masks import make_identity
                ident = sbuf.tile([P, P], F16)
                make_identity(nc, ident[:])

            # W2 shard resident in SBUF.
            w2_sb = sbuf.tile([P, SP], F16)
            nc.sync.dma_start(out=w2_sb[:], in_=w2s[:])

            hidT = []        # per-chunk (sbuf tile, dram cc_out)
            ebufs = [None] * BT
            lsum = sbuf.tile([P, BT], F32)

            def prologue_chunk(k):
                ct = CHUNKS[k]
                t0 = CH_START[k]
                hch = hidp.tile([P, ct * P], F16, tag=f"hch{ct}")
                for tt in range(ct):
                    t = t0 + tt
                    if USE_DMA_GATHER and GATHER_TRANSPOSE:
                        # gath[d, c*128+p] = W1s[locidx[t*128+p, c], d]
                        gath = gathp.tile([P, C * P], F16, tag="gath")
                        nc.gpsimd.dma_gather(
                            gath[:].rearrange("d (a n) -> d a n", a=1),
                            w1s[:],
                            idx_sb[:, t * IPT:(t + 1) * IPT],
                            C * P,
                            C * P,
                            D,
                            transpose=True,
                            single_packet=GATHER_SINGLE_PACKET,
                        )
                        hidf = hidp.tile([P, P], F32, tag="hidf")
                        nc.vector.tensor_reduce(
                            out=hidf[:],
                            in_=gath[:].rearrange("d (c p) -> d p c", c=C),
                            axis=mybir.AxisListType.X,
                            op=mybir.AluOpType.add,
                        )
                        # context mean folded in here (x 1/10), cast to f16
                        nc.vector.tensor_scalar_mul(
                            hch[:, tt * P:(tt + 1) * P], hidf[:], 1.0 / C)
                    elif USE_DMA_GATHER:
                        # non-transposed: gath[p, c*D:] = W1s[locidx[..]]
                        gath = gathp.tile([P, C * D], F16, tag="gath")
                        nc.gpsimd.dma_gather(
                            gath[:].rearrange("p (c d) -> p c d", c=C),
                            w1s[:],
                            idx_sb[:, t * IPT:(t + 1) * IPT],
                            C * P,
                            C * P,
                            D,
                            transpose=False,
                            single_packet=GATHER_SINGLE_PACKET,
                        )
                        hidf = hidp.tile([P, D], F32, tag="hidf")
                        nc.vector.tensor_reduce(
                            out=hidf[:],
                            in_=gath[:].rearrange("p (c d) -> p d c", c=C),
                            axis=mybir.AxisListType.X,
                            op=mybir.AluOpType.add,
                        )
                        hid16 = hidp.tile([P, D], F16, tag="hid16")
                        nc.vector.tensor_scalar_mul(hid16[:], hidf[:],
                                                    1.0 / C)
                        tp = psum.tile([P, 2 * GRP], F16, tag="mm")
                        nc.tensor.transpose(out=tp[:, :P], in_=hid16[:],
                                            identity=ident[:])
                        nc.vector.tensor_copy(hch[:, tt * P:(tt + 1) * P],
                                              tp[:, :P])
                    else:
                        gath = gathp.tile([P, C * D], F16, tag="gath")
                        for c in range(C):
                            j = t * C + c
                            nc.gpsimd.indirect_dma_start(
                                out=gath[:, c * D:(c + 1) * D],
                                out_offset=None,
                                in_=w1s[:],
                                in_offset=bass.IndirectOffsetOnAxis(
                                    ap=idx_sb[:, j:j + 1], axis=0),
                            )
                        hidf = hidp.tile([P, D], F32, tag="hidf")
                        nc.vector.tensor_reduce(
                            out=hidf[:],
                            in_=gath[:].rearrange("p (c d) -> p d c", c=C),
                            axis=mybir.AxisListType.X,
                            op=mybir.AluOpType.add,
                        )
                        hid16 = hidp.tile([P, D], F16, tag="hid16")
                        nc.vector.tensor_scalar_mul(hid16[:], hidf[:],
                                                    1.0 / C)
                        tp = psum.tile([P, 2 * GRP], F16, tag="mm")
                        nc.tensor.transpose(out=tp[:, :P], in_=hid16[:],
                                            identity=ident[:])
                        nc.vector.tensor_copy(hch[:, tt * P:(tt + 1) * P],
                                              tp[:, :P])
                cc_in = dram.tile([P, ct * P], F16)
                cc_out = dram.tile(
                    [P, ct * P], F16,
                    addr_space="Shared" if n_cores > 1 else "Local")
                nc.gpsimd.dma_start(out=cc_in[:], in_=hch[:])
                if n_cores > 1:
                    nc.gpsimd.collective_compute(
                        "AllReduce", mybir.AluOpType.add, replica_groups=rg,
                        ins=[cc_in[:]], outs=[cc_out[:]],
                    )
                else:
                    nc.gpsimd.dma_start(out=cc_out[:], in_=cc_in[:])
                ht = sbuf.tile([P, ct * P], F16, name=f"hidT{k}")
                hidT.append((ht, cc_out))

            def read_hidT(k):
                ht, cc_out = hidT[k]
                nc.sync.dma_start(out=ht[:], in_=cc_out[:])

            def pass1_tile(t):
                for k in range(NCH):
                    if CH_START[k] <= t < CH_START[k] + CHUNKS[k]:
                        break
                lhsT = hidT[k][0][:, (t - CH_START[k]) * P:
                                  (t - CH_START[k] + 1) * P]
                stag = stagp.tile([P, SP], F16, tag="stag")
                eb = ebufp.tile([P, SP], BF16, tag="eb")
                ebufs[t] = eb
                sums = hidp.tile([P, len(GROUPS)], F32, tag="sums")
                for gi, (g0, gw) in enumerate(GROUPS):
                    ps = psum.tile([P, GRP], F32, tag="mm")
                    for s0 in range(0, gw, MMN):
                        w = min(MMN, gw - s0)
                        nc.tensor.matmul(
                            out=ps[:, s0:s0 + w], lhsT=lhsT,
                            rhs=w2_sb[:, g0 + s0:g0 + s0 + w],
                            start=True, stop=True)
                    nc.scalar.activation(
                        out=eb[:, g0:g0 + gw], in_=ps[:, :gw], func=AF.Exp,
                        accum_out=sums[:, gi:gi + 1])
                    if gi in ACT_COPY_GROUPS:
                        nc.scalar.copy(stag[:, g0:g0 + gw], ps[:, :gw])
                    else:
                        nc.vector.tensor_copy(stag[:, g0:g0 + gw],
                                              ps[:, :gw])
                nc.vector.tensor_reduce(
                    out=lsum[:, t:t + 1], in_=sums[:],
                    axis=mybir.AxisListType.X, op=mybir.AluOpType.add)
                nc.sync.dma_start(
                    out=logits_s[t * P:(t + 1) * P, :], in_=stag[:, :S])

            gsums = [None] * NSC

            def sum_allreduce(j):
                h0 = j * SCH
                cc_s_in = dram.tile([P, SCH], F32, name=f"ccsi{j}")
                cc_s_out = dram.tile(
                    [P, SCH], F32, name=f"ccso{j}",
                    addr_space="Shared" if n_cores > 1 else "Local")
                nc.gpsimd.dma_start(out=cc_s_in[:],
                                    in_=lsum[:, h0:h0 + SCH])
                if n_cores > 1:
                    nc.gpsimd.collective_compute(
                        "AllReduce", mybir.AluOpType.add, replica_groups=rg,
                        ins=[cc_s_in[:]], outs=[cc_s_out[:]],
                    )
                else:
                    nc.gpsimd.dma_start(out=cc_s_out[:], in_=cc_s_in[:])
                gsum = sbuf.tile([P, SCH], F32, name=f"gsum{j}")
                nc.gpsimd.dma_start(out=gsum[:], in_=cc_s_out[:])
                gsums[j] = gsum

            def scale_chunk(j):
                zinv = sbuf.tile([P, SCH], F32, name=f"zinv{j}")
                nc.vector.reciprocal(zinv[:], gsums[j][:])
                for tt in range(SCH):
                    t = j * SCH + tt
                    eb = ebufs[t]
                    nc.vector.tensor_scalar_mul(
                        eb[:], eb[:], zinv[:, tt:tt + 1])

            def soft_dma_chunk(j):
                eng = nc.scalar if SOFT_RING_SCALAR else nc.sync
                for tt in range(SCH):
                    t = j * SCH + tt
                    eng.dma_start(
                        out=soft_s[t * P:(t + 1) * P, :],
                        in_=ebufs[t][:, :S])

            # ---- software-pipelined emission ----
            prologue_chunk(0)
            prologue_chunk(1)
            k = 0
            for t in range(BT):
                if k < NCH and t == CH_START[k]:
                    if k + 2 < NCH:
                        prologue_chunk(k + 2)
                    read_hidT(k)
                    k += 1
                pass1_tile(t)
                if t % SCH == SCH - 1:
                    sum_allreduce(t // SCH)
                elif t % SCH == 0 and t >= SCH:
                    # first tile of sum-chunk j: scale chunk j-1 (its
                    # AllReduce was emitted one chunk ago) and queue the
                    # softmax writes behind this tile's ACT work.
                    scale_chunk(t // SCH - 1)
                    soft_dma_chunk(t // SCH - 1)
            scale_chunk(NSC - 1)
            soft_dma_chunk(NSC - 1)

    nc.compile()
    return nc


def make_in_maps(inputs: np.ndarray, W1: np.ndarray, W2: np.ndarray,
                 n_cores: int = M):
    idx = np.asarray(inputs).astype(np.int64)
    W1 = np.asarray(W1, dtype=np.float32)
    W2 = np.asarray(W2, dtype=np.float32)
    in_maps = []
    for m in range(n_cores):
        lo = m * S
        loc = idx - lo
        idxm = np.where((loc >= 0) & (loc < S), loc, S)
        w1m = np.empty((S + 1, D), np.float16)
        w1m[:S] = W1[lo:lo + S]
        w1m[S] = 0
        w2m = np.zeros((P, SP), np.float16)
        w2m[:, :S] = W2[:, lo:lo + S]
        if USE_DMA_GATHER:
            # dma_gather linear order for tile t: i = c*128 + p, wrapped as
            # [i % 16, t*80 + i//16]  ->  [p % 16, t*80 + c*8 + p//16]
            lr = idxm.astype(np.int16).reshape(BT, 8, 16, C)
            arr = np.ascontiguousarray(
                lr.transpose(2, 0, 3, 1).reshape(16, BT * C * 8))
            idx16 = np.tile(arr, (8, 1))              # replicate to 128 parts
            in_maps.append({"w1s": w1m, "w2s": w2m, "idx16": idx16})
        else:
            idxm32 = np.ascontiguousarray(
                idxm.astype(np.int32).reshape(BT, P, C)
                .transpose(1, 0, 2).reshape(P, BT * C))
            in_maps.append({"w1s": w1m, "w2s": w2m, "idxs": idxm32})
    return in_maps


_NC_CACHE = {}


def kernel(inputs: np.ndarray, W1: np.ndarray, W2: np.ndarray):
    if "nc" not in _NC_CACHE:
        _NC_CACHE["nc"] = build_nc(M)
    nc = _NC_CACHE["nc"]
    in_maps = make_in_maps(inputs, W1, W2, M)
    res = bass_utils.run_bass_kernel_spmd(nc, in_maps, core_ids=list(range(M)))
    logits = np.empty((B, V), np.float32)
    soft = np.empty((B, V), np.float32)
    for m in range(M):
        logits[:, m * S:(m + 1) * S] = np.asarray(
            res.results[m]["logits_s"]).astype(np.float32)
        soft[:, m * S:(m + 1) * S] = np.asarray(
            res.results[m]["soft_s"]).astype(np.float32)
    return logits, soft


# revision 24
# speedup vs baseline: 1.3599x; 1.3599x over previous
"""CBOW model (embedding gather -> mean -> logits -> softmax) on 8 Trainium2
NeuronCores.

Sharding (vocab/model parallel for the matmul + softmax, per the hint; the
gather is batch-sharded against a replicated W1 to keep the random-row DMA
traffic tiny and off the critical path):
  - W2 is sharded along vocab: core m owns columns [m*12500, (m+1)*12500)
    (fp16, padded to 12544).  W1 is replicated (fp16) and each core gathers
    only ITS two batch tiles (256 rows x 10 ctx) -> 0.65 MB of random reads
    per core instead of 5.2 MB.  A 2-chunk AllGather (even tiles / odd
    tiles) gives every core the full transposed hidden [D, 2048] ~50us in.
  - Softmax in a SINGLE matmul/exp pass per batch tile: matmul chunks
    produce f32 logits in PSUM; ACT computes exp -> SBUF (bf16, kept
    resident in one of 6 rotating buffers) with fused per-row partial sums
    (accum_out); DVE/ACT copy the f32 logits to f16 per-group staging tiles
    that DMA straight out.  Per 2-tile chunk the row sums are AllReduced
    (the collective stream is otherwise idle after the AllGathers, and a
    dummy warmup collective absorbs the first-collective latency); 1/Z
    comes from DVE reciprocal and the resident exp values are scaled in
    place (DVE 4x tensor_scalar, per-partition scalar) and DMA'd out as
    softmax (bf16) on the ACT HWDGE ring, two tiles behind the compute
    wavefront.  No second matmul or exp pass.  Max-subtraction is not
    needed: |logit| < 40 always, exp fits bf16 comfortably.
"""

import numpy as np

import concourse.bass as bass
import concourse.mybir as mybir
import concourse.tile as tile
from concourse import bacc
from concourse.masks import make_identity
import concourse.bass_utils as bass_utils

# Problem shape (hardcoded; matches the CBOW reference).
V = 100000      # vocab
D = 128         # embed dim
B = 2048        # batch
C = 10          # context positions
M = 8           # cores
S = V // M      # vocab shard per core = 12500
SP = 12544      # shard padded so every matmul chunk is >= 256 wide
P = 128         # partitions
BT = B // P     # batch tiles = 16
TPC = BT // M   # batch tiles gathered per core = 2
MMN = 512       # max moving free dim per matmul (one PSUM bank, f32)
GRP = 2048      # vocab columns per PSUM group (4 banks)

F32 = mybir.dt.float32
F16 = mybir.dt.float16
BF16 = mybir.dt.bfloat16
I32 = mybir.dt.int32
AF = mybir.ActivationFunctionType

# (start, width) vocab-column groups per core; width <= GRP.  The pad columns
# beyond S are never computed past the matmul (exp/copy/DMA use gwS).
GROUPS = [(g0, min(GRP, SP - g0)) for g0 in range(0, SP, GRP)]
# groups whose PSUM->SBUF logits cast runs on the scalar engine instead of
# DVE, to balance the two engines (ACT also does every group's exp).
ACT_COPY_GROUPS = {len(GROUPS) - 1, len(GROUPS) - 2}
SCH = 2          # batch tiles per softmax-sum AllReduce
NSC = BT // SCH
EBUFS = 6        # resident exp buffers (SBUF budget-limited)


def build_nc(n_cores: int = M):
    nc = bacc.Bacc("TRN2", target_bir_lowering=False, debug=False,
                   num_devices=n_cores)

    w1f = nc.dram_tensor("w1f", [V, D], F16, kind="ExternalInput")
    w2s = nc.dram_tensor("w2s", [P, SP], F16, kind="ExternalInput")
    idxs = nc.dram_tensor("idxs", [P, TPC * C], I32, kind="ExternalInput")
    logits_s = nc.dram_tensor("logits_s", [B, S], F16, kind="ExternalOutput")
    soft_s = nc.dram_tensor("soft_s", [B, S], BF16, kind="ExternalOutput")

    rg = [list(range(n_cores))]
    shared = "Shared" if n_cores > 1 else "Local"

    with tile.TileContext(nc) as tc:
        with tc.tile_pool(name="sbuf", bufs=1) as sbuf, \
             tc.tile_pool(name="gathp", bufs=2) as gathp, \
             tc.tile_pool(name="hidp", bufs=2) as hidp, \
             tc.tile_pool(name="stagp", bufs=4) as stagp, \
             tc.tile_pool(name="ebufp", bufs=EBUFS) as ebufp, \
             tc.tile_pool(name="psum", bufs=2, space="PSUM") as psum, \
             tc.tile_pool(name="dram", bufs=1, space="DRAM") as dram:
            idx_sb = sbuf.tile([P, TPC * C], I32)
            nc.sync.dma_start(out=idx_sb[:], in_=idxs[:])

            ident = sbuf.tile([P, P], F16)
            make_identity(nc, ident[:])

            # W2 shard resident in SBUF for the whole kernel.
            w2_sb = sbuf.tile([P, SP], F16)
            nc.sync.dma_start(out=w2_sb[:], in_=w2s[:])

            if n_cores > 1:
                # Warm up the collectives stream with a dummy tiny AllReduce
                # so the first real collective doesn't pay the ~45us
                # first-collective latency.
                warm_sb = sbuf.tile([P, 2], F32)
                nc.gpsimd.memset(warm_sb[:], 0.0)
                warm_in = dram.tile([P, 2], F32, name="warm_in")
                warm_out = dram.tile([P, 2], F32, name="warm_out",
                                     addr_space="Shared")
                nc.gpsimd.dma_start(out=warm_in[:], in_=warm_sb[:])
                nc.gpsimd.collective_compute(
                    "AllReduce", mybir.AluOpType.add, replica_groups=rg,
                    ins=[warm_in[:]], outs=[warm_out[:]],
                )

            # ---- gather my TPC batch tiles, build transposed hidden ----
            hch = sbuf.tile([P, TPC * P], F16)   # [D, tt*128+p]
            for tt in range(TPC):
                gath = gathp.tile([P, C * D], F16, tag="gath")
                for c in range(C):
                    j = tt * C + c
                    nc.gpsimd.indirect_dma_start(
                        out=gath[:, c * D:(c + 1) * D],
                        out_offset=None,
                        in_=w1f[:],
                        in_offset=bass.IndirectOffsetOnAxis(
                            ap=idx_sb[:, j:j + 1], axis=0),
                    )
                hidf = hidp.tile([P, D], F32, tag="hidf")
                nc.vector.tensor_reduce(
                    out=hidf[:],
                    in_=gath[:].rearrange("p (c d) -> p d c", c=C),
                    axis=mybir.AxisListType.X,
                    op=mybir.AluOpType.add,
                )
                hid16 = hidp.tile([P, D], F16, tag="hid16")
                # context mean folded in here (x 1/10)
                nc.vector.tensor_scalar_mul(hid16[:], hidf[:], 1.0 / C)
                tp = psum.tile([P, 2 * GRP], F16, tag="mm")
                nc.tensor.transpose(out=tp[:, :P], in_=hid16[:],
                                    identity=ident[:])
                nc.vector.tensor_copy(hch[:, tt * P:(tt + 1) * P],
                                      tp[:, :P])

            # ---- AllGather the hidden in 2 chunks (even tiles, odd tiles).
            # Core m's chunk lands in block m: cc_out[(m*P+d), p] =
            # hidden[d, (TPC*m + tt)*128 + p].
            assert n_cores in (1, M)
            hidT = []
            for tt in range(TPC):
                cc_in = dram.tile([P, P], F16, name=f"hag_in{tt}")
                cc_out = dram.tile([M * P, P], F16, name=f"hag_out{tt}",
                                   addr_space=shared)
                nc.gpsimd.dma_start(out=cc_in[:],
                                    in_=hch[:, tt * P:(tt + 1) * P])
                if n_cores > 1:
                    nc.gpsimd.collective_compute(
                        "AllGather", mybir.AluOpType.bypass,
                        replica_groups=rg,
                        ins=[cc_in[:]], outs=[cc_out[:]],
                    )
                else:
                    # debug build: only block 0 is real; tiles 2..15 garbage
                    nc.gpsimd.dma_start(out=cc_out[:P, :], in_=cc_in[:])
                ht = sbuf.tile([P, M * P], F16, name=f"hidT{tt}")
                hidT.append((ht, cc_out))

            def read_hidT(tt):
                ht, cc_out = hidT[tt]
                nc.sync.dma_start(
                    out=ht[:].rearrange("d (m p) -> d m p", m=M),
                    in_=cc_out[:].rearrange("(m d) p -> d m p", m=M))

            def lhsT_of(t):
                # tile t = TPC*m + tt -> chunk tt, block m
                return hidT[t % TPC][0][:, (t // TPC) * P:(t // TPC + 1) * P]

            ebufs = [None] * BT
            lsum = sbuf.tile([P, BT], F32)
            gsums = [None] * NSC

            def pass1_tile(t):
                lhsT = lhsT_of(t)
                eb = ebufp.tile([P, SP], BF16, tag="eb")
                ebufs[t] = eb
                sums = hidp.tile([P, len(GROUPS)], F32, tag="sums")
                for gi, (g0, gw) in enumerate(GROUPS):
                    gwS = min(gw, S - g0)   # drop the zero-pad columns
                    ps = psum.tile([P, GRP], F32, tag="mm")
                    for s0 in range(0, gw, MMN):
                        w = min(MMN, gw - s0)
                        nc.tensor.matmul(
                            out=ps[:, s0:s0 + w], lhsT=lhsT,
                            rhs=w2_sb[:, g0 + s0:g0 + s0 + w],
                            start=True, stop=True)
                    nc.scalar.activation(
                        out=eb[:, g0:g0 + gwS], in_=ps[:, :gwS], func=AF.Exp,
                        accum_out=sums[:, gi:gi + 1])
                    stag = stagp.tile([P, GRP], F16, tag="stag")
                    if gi in ACT_COPY_GROUPS:
                        nc.scalar.copy(stag[:, :gwS], ps[:, :gwS])
                    else:
                        nc.vector.tensor_copy(stag[:, :gwS], ps[:, :gwS])
                    nc.sync.dma_start(
                        out=logits_s[t * P:(t + 1) * P, g0:g0 + gwS],
                        in_=stag[:, :gwS])
                nc.vector.tensor_reduce(
                    out=lsum[:, t:t + 1], in_=sums[:],
                    axis=mybir.AxisListType.X, op=mybir.AluOpType.add)

            def sum_allreduce(j):
                h0 = j * SCH
                cc_s_in = dram.tile([P, SCH], F32, name=f"ccsi{j}")
                cc_s_out = dram.tile([P, SCH], F32, name=f"ccso{j}",
                                     addr_space=shared)
                nc.gpsimd.dma_start(out=cc_s_in[:],
                                    in_=lsum[:, h0:h0 + SCH])
                if n_cores > 1:
                    nc.gpsimd.collective_compute(
                        "AllReduce", mybir.AluOpType.add, replica_groups=rg,
                        ins=[cc_s_in[:]], outs=[cc_s_out[:]],
                    )
                else:
                    nc.gpsimd.dma_start(out=cc_s_out[:], in_=cc_s_in[:])
                gsum = sbuf.tile([P, SCH], F32, name=f"gsum{j}")
                nc.gpsimd.dma_start(out=gsum[:], in_=cc_s_out[:])
                gsums[j] = gsum

            def scale_chunk(j):
                zinv = sbuf.tile([P, SCH], F32, name=f"zinv{j}")
                nc.vector.reciprocal(zinv[:], gsums[j][:])
                for tt in range(SCH):
                    t = j * SCH + tt
                    eb = ebufs[t]
                    nc.vector.tensor_scalar_mul(
                        eb[:, :S], eb[:, :S], zinv[:, tt:tt + 1])

            def soft_dma_chunk(j):
                for tt in range(SCH):
                    t = j * SCH + tt
                    nc.scalar.dma_start(
                        out=soft_s[t * P:(t + 1) * P, :],
                        in_=ebufs[t][:, :S])

            # ---- emission ----
            read_hidT(0)
            for t in range(BT):
                pass1_tile(t)
                if t == 0 and TPC > 1:
                    read_hidT(1)
                if t % SCH == 1:
                    sum_allreduce(t // SCH)
                    if t >= 3:
                        # scale chunk j-1: its AllReduce was emitted a full
                        # chunk ago, so the wait is nearly always satisfied.
                        scale_chunk(t // SCH - 1)
                        soft_dma_chunk(t // SCH - 1)
            scale_chunk(NSC - 1)
            soft_dma_chunk(NSC - 1)

    nc.compile()
    return nc


def make_in_maps(inputs: np.ndarray, W1: np.ndarray, W2: np.ndarray,
                 n_cores: int = M):
    idx = np.asarray(inputs).astype(np.int64)
    w1m = np.asarray(W1, dtype=np.float32).astype(np.float16)
    W2 = np.asarray(W2, dtype=np.float32)
    in_maps = []
    for m in range(n_cores):
        lo = m * S
        # tiles TPC*m .. TPC*m+TPC-1 of the batch, global vocab indices
        rows = idx[TPC * m * P:TPC * (m + 1) * P]          # [TPC*128, C]
        idxm = np.ascontiguousarray(
            rows.astype(np.int32).reshape(TPC, P, C)
            .transpose(1, 0, 2).reshape(P, TPC * C))
        w2m = np.zeros((P, SP), np.float16)
        w2m[:, :S] = W2[:, lo:lo + S]
        in_maps.append({"w1f": w1m, "w2s": w2m, "idxs": idxm})
    return in_maps


_NC_CACHE = {}


def kernel(inputs: np.ndarray, W1: np.ndarray, W2: np.ndarray):
    if "nc" not in _NC_CACHE:
        _NC_CACHE["nc"] = build_nc(M)
    nc = _NC_CACHE["nc"]
    in_maps = make_in_maps(inputs, W1, W2, M)
    res = bass_utils.run_bass_kernel_spmd(nc, in_maps, core_ids=list(range(M)))
    logits = np.empty((B, V), np.float32)
    soft = np.empty((B, V), np.float32)
    for m in range(M):
        logits[:, m * S:(m + 1) * S] = np.asarray(
            res.results[m]["logits_s"]).astype(np.float32)
        soft[:, m * S:(m + 1) * S] = np.asarray(
            res.results[m]["soft_s"]).astype(np.float32)
    return logits, soft
